# revision 1
# baseline (speedup 1.0000x reference)
"""BERT self-attention (B=8, S=1024, D=1024, H=16, Dh=64) on 8 NeuronCores.

Sharding: pure data parallel — core b handles batch element b (B == n_cores),
qkv_weight replicated. No collectives.

Per-core dataflow (all matmuls bf16 with fp32 PSUM accumulation):
  1. X [S,D] loaded, cast to bf16, PE-transposed into X^T [D,S].
  2. W loaded as per-head-pair column slices (overlaps with compute),
     Q^T,K^T computed as [features, S]; V computed as [S, features] and laid
     out as [S, H*(Dh+1)] where each head's 65th column carries exp(mask):
     softmax(s + m) == exp(s)*exp(m) normalized, so the additive mask is an
     exact per-key row scaling of V', and the ones-ish column makes the PV
     matmul emit softmax denominators for free.
  3. Per head: scores^T [S_k,S_q] = (K^T chunk).T @ Q^T;  ACT computes
     exp(0.125*s) PSUM->SBUF(bf16);  ctx'^T [65,S_q] = V'.T @ expS^T;
     PE-transpose back to [S_q,65], multiply cols 0..63 by 1/col64.
  4. ctx assembled [S, D] fp32, DMA'd out.

No max-subtraction in softmax: scores*scale is bounded (|x| <~ 4 for this
problem's scale) and exp runs in fp32 on ACT.
"""

import sys

import numpy as np

_REPO = "/opt/trn_rl_repo"
if _REPO not in sys.path:
    sys.path.insert(0, _REPO)

B, S, D, H, DH = 8, 1024, 1024, 16, 64
P = 128
NS = S // P          # seq tiles
NK = D // P          # contraction tiles
NHP = H // 2         # head pairs
NQ = 2               # 512-wide S_q chunks
QC = S // NQ         # 512
SCALE = 1.0 / 8.0    # 1/sqrt(DH)
VW = DH + 1          # V' width per head (extra denominator column)

_NC_CACHE = {}


def _build_nc():
    import concourse.bass as bass
    import concourse.tile as tile
    from concourse import bacc, mybir
    from concourse.masks import make_identity
    from contextlib import ExitStack

    f32 = mybir.dt.float32
    bf16 = mybir.dt.bfloat16
    Exp = mybir.ActivationFunctionType.Exp

    nc = bacc.Bacc("TRN2", target_bir_lowering=False, debug=False)
    x_d = nc.declare_dram_parameter("x", [S, D], f32, isOutput=False)
    w_d = nc.declare_dram_parameter("w", [D, 3 * D], f32, isOutput=False)
    m_d = nc.declare_dram_parameter("m", [S], f32, isOutput=False)
    o_d = nc.declare_dram_parameter("o", [S, D], f32, isOutput=True)

    with tile.TileContext(nc) as tc, ExitStack() as es:
        const = es.enter_context(tc.tile_pool(name="const", bufs=1))
        maskp = es.enter_context(tc.tile_pool(name="maskp", bufs=NS))
        xtp = es.enter_context(tc.tile_pool(name="xtp", bufs=NK))
        vp = es.enter_context(tc.tile_pool(name="vp", bufs=NS))
        ctxp = es.enter_context(tc.tile_pool(name="ctxp", bufs=NS))
        stage = es.enter_context(tc.tile_pool(name="stage", bufs=2))
        wbfp = es.enter_context(tc.tile_pool(name="wbfp", bufs=2))
        qktp = es.enter_context(tc.tile_pool(name="qktp", bufs=2))
        esp = es.enter_context(tc.tile_pool(name="esp", bufs=2 * NK))
        ctp = es.enter_context(tc.tile_pool(name="ctp", bufs=4))
        smallp = es.enter_context(tc.tile_pool(name="smallp", bufs=4))
        psA = es.enter_context(tc.tile_pool(name="psA", bufs=4, space="PSUM"))
        psC = es.enter_context(tc.tile_pool(name="psC", bufs=2, space="PSUM"))
        psT = es.enter_context(tc.tile_pool(name="psT", bufs=2, space="PSUM"))

        id_bf = const.tile([P, P], bf16, name="id_bf")
        make_identity(nc, id_bf)
        id_f32 = const.tile([P, P], f32, name="id_f32")
        make_identity(nc, id_f32)
        ones16 = const.tile([P, H], bf16, name="ones16")
        nc.vector.memset(ones16, 1.0)

        # mask -> exp(mask) per seq tile, [128,1] per-partition scalars
        em = []
        for st in range(NS):
            msk = maskp.tile([P, 1], f32, name=f"msk{st}", tag="msk")
            nc.sync.dma_start(
                out=msk,
                in_=m_d[st * P:(st + 1) * P].rearrange("(p o) -> p o", o=1),
            )
            emt = maskp.tile([P, 1], f32, name=f"em{st}", tag="em")
            nc.scalar.activation(emt, msk, Exp)
            em.append(emt)

        # persistent tensors
        xt = [xtp.tile([P, S], bf16, name=f"xt{j}", tag="xt") for j in range(NK)]
        v_sb = [vp.tile([P, H * VW], bf16, name=f"v{st}", tag="v") for st in range(NS)]
        ctx_sb = [ctxp.tile([P, D], f32, name=f"cx{st}", tag="cx") for st in range(NS)]

        # V' denominator columns = exp(mask) per key row
        for st in range(NS):
            vcols = v_sb[st].rearrange("p (h c) -> p h c", h=H)[:, :, DH]
            nc.scalar.mul(vcols, ones16, em[st])

        # X load, cast to bf16, PE-transpose into X^T
        for i in range(NS):
            xf = stage.tile([P, D], f32, name=f"xf{i}", tag="xf")
            nc.sync.dma_start(out=xf, in_=x_d[i * P:(i + 1) * P, :])
            xb = stage.tile([P, D], bf16, name=f"xb{i}", tag="xb")
            nc.vector.tensor_copy(xb, xf)
            for j in range(NK):
                pst = psT.tile([P, P], bf16, name=f"px{i}_{j}", tag="pst")
                nc.tensor.transpose(pst, xb[:, j * P:(j + 1) * P], id_bf)
                nc.vector.tensor_copy(xt[j][:, i * P:(i + 1) * P], pst)

        for hp in range(NHP):
            # W column slices for this head pair: q, k, v
            wbf = []
            for t, base in enumerate((hp * P, D + hp * P, 2 * D + hp * P)):
                wf = stage.tile([P, NK, P], f32, name=f"wf{hp}_{t}", tag="wf")
                nc.sync.dma_start(
                    out=wf,
                    in_=w_d[:, base:base + P].rearrange("(kt p) c -> p kt c", p=P),
                )
                wb = wbfp.tile([P, NK, P], bf16, name=f"wb{hp}_{t}", tag=f"wb{t}")
                nc.vector.tensor_copy(wb, wf)
                wbf.append(wb)

            # Q^T, K^T for the pair: [128 feats, S]
            qt_t = qktp.tile([P, S], bf16, name=f"qt{hp}", tag="qt")
            kt_t = qktp.tile([P, S], bf16, name=f"kt{hp}", tag="kt")
            for wsel, dest in ((0, qt_t), (1, kt_t)):
                for n in range(NQ):
                    ps = psA.tile([P, QC], f32, name=f"pq{hp}_{wsel}_{n}", tag="psA")
                    for k in range(NK):
                        nc.tensor.matmul(
                            ps,
                            wbf[wsel][:, k, :],
                            xt[k][:, n * QC:(n + 1) * QC],
                            start=(k == 0),
                            stop=(k == NK - 1),
                        )
                    nc.vector.tensor_copy(dest[:, n * QC:(n + 1) * QC], ps)

            # V for the pair: [S, 128 feats], scaled by exp(mask), strided into v_sb
            for st in range(NS):
                ps = psA.tile([P, P], f32, name=f"pv{hp}_{st}", tag="psA")
                for k in range(NK):
                    nc.tensor.matmul(
                        ps,
                        xt[k][:, st * P:(st + 1) * P],
                        wbf[2][:, k, :],
                        start=(k == 0),
                        stop=(k == NK - 1),
                    )
                vdst = v_sb[st].rearrange("p (h c) -> p h c", h=H)[:, 2 * hp:2 * hp + 2, 0:DH]
                vsrc = ps.rearrange("p (h c) -> p h c", h=2)
                nc.scalar.mul(vdst, vsrc, em[st])

            for h2 in range(2):
                h = 2 * hp + h2
                hs = h2 * DH
                esb = [esp.tile([P, S], bf16, name=f"e{h}_{k}", tag="es") for k in range(NK)]
                # scores^T [S_k, S_q] then exp
                for k in range(NK):
                    for qn in range(NQ):
                        ps = psA.tile([P, QC], f32, name=f"s{h}_{k}_{qn}", tag="psA")
                        nc.tensor.matmul(
                            ps,
                            kt_t[hs:hs + DH, k * P:(k + 1) * P],
                            qt_t[hs:hs + DH, qn * QC:(qn + 1) * QC],
                            start=True,
                            stop=True,
                        )
                        nc.scalar.activation(
                            esb[k][:, qn * QC:(qn + 1) * QC], ps, Exp, scale=SCALE
                        )
                # ctx'^T [65, S_q] = V'.T @ expS^T, then transpose + normalize
                for qn in range(NQ):
                    psc = psC.tile([VW, QC], f32, name=f"c{h}_{qn}", tag="psC")
                    for k in range(NK):
                        nc.tensor.matmul(
                            psc,
                            v_sb[k][:, h * VW:(h + 1) * VW],
                            esb[k][:, qn * QC:(qn + 1) * QC],
                            start=(k == 0),
                            stop=(k == NK - 1),
                        )
                    ct = ctp.tile([VW, QC], f32, name=f"ct{h}_{qn}", tag="ct")
                    nc.vector.tensor_copy(ct, psc)
                    for qs in range(QC // P):
                        qi = qn * (QC // P) + qs
                        pst = psT.tile([P, P], f32, name=f"pt{h}_{qi}", tag="pst")
                        nc.tensor.transpose(
                            pst[:, 0:VW], ct[:, qs * P:(qs + 1) * P], id_f32[0:VW, 0:VW]
                        )
                        rec = smallp.tile([P, 1], f32, name=f"r{h}_{qi}", tag="rec")
                        nc.vector.reciprocal(rec, pst[:, DH:DH + 1])
                        nc.scalar.mul(
                            ctx_sb[qi][:, h * DH:(h + 1) * DH], pst[:, 0:DH], rec
                        )

        for st in range(NS):
            nc.sync.dma_start(out=o_d[st * P:(st + 1) * P, :], in_=ctx_sb[st])

    nc.finalize()
    return nc


def _get_nc():
    if "nc" not in _NC_CACHE:
        _NC_CACHE["nc"] = _build_nc()
    return _NC_CACHE["nc"]


def _run(hidden_states, attention_mask, qkv_weight, trace=False, **trace_kw):
    from concourse.bass_utils import run_bass_kernel_spmd

    nc = _get_nc()
    hidden = np.ascontiguousarray(np.asarray(hidden_states, dtype=np.float32))
    mask = np.ascontiguousarray(
        np.asarray(attention_mask, dtype=np.float32).reshape(B, S)
    )
    w = np.ascontiguousarray(np.asarray(qkv_weight, dtype=np.float32))
    in_maps = [
        {"x": hidden[b], "w": w, "m": mask[b]} for b in range(B)
    ]
    res = run_bass_kernel_spmd(nc, in_maps, list(range(B)), trace=trace, **trace_kw)
    out = np.stack([np.asarray(res.results[b]["o"]) for b in range(B)], axis=0)
    return out.astype(np.float32), res


def kernel(hidden_states, attention_mask, qkv_weight):
    out, _ = _run(hidden_states, attention_mask, qkv_weight, trace=False)
    return out

